# revision 1
# baseline (speedup 1.0000x reference)
"""Grouped linear (MoE routing) kernel for 8 Trainium2 NeuronCores.

out[t] = input_tokens[t] @ weight[expert_assignments[t]].T

Strategy (expert-parallel): the host groups tokens by expert (argsort),
pads every group to a common capacity C (multiple of 128), and core e
computes the dense GEMM  Y_e = X_e @ W_e.T  for expert e.  The host then
scatters rows back to the original token order.

Per-core Bass/Tile kernel: X is staged in DRAM pre-transposed ([in, C])
so the contraction dim lands on SBUF partitions; W is staged as W.T
([in, out]).  The full W.T (16 MB) is resident in SBUF; token tiles of
128 stream through.  Matmuls run in float32r (fp32 data, reduced-
precision single-pass multiply) which runs at full PE rate.
"""

import numpy as np

import concourse.mybir as mybir
import concourse.tile as tile
from concourse import bacc
from concourse.bass_utils import run_bass_kernel_spmd

NUM_EXPERTS = 8
D_IN = 2048
D_OUT = 2048
P = 128
KO = D_IN // P  # 16 contraction subtiles
NB = D_OUT // 512  # 4 psum banks per token tile

MM_DT = mybir.dt.float32r

_nc_cache = {}


def _build_nc(C: int):
    """Bass module: y[C, D_OUT] = xT.T @ wT  (xT: [D_IN, C], wT: [D_IN, D_OUT])."""
    nc = bacc.Bacc("TRN2", target_bir_lowering=False, debug=False,
                   num_devices=NUM_EXPERTS)
    xT = nc.dram_tensor("xT", [D_IN, C], MM_DT, kind="ExternalInput")
    wT = nc.dram_tensor("wT", [D_IN, D_OUT], MM_DT, kind="ExternalInput")
    y = nc.dram_tensor("y", [C, D_OUT], mybir.dt.float32, kind="ExternalOutput")

    M_TILES = C // P
    xT3 = xT.rearrange("(ko p) m -> p ko m", p=P)

    with tile.TileContext(nc) as tc:
        with (
            tc.tile_pool(name="w", bufs=1) as wpool,
            tc.tile_pool(name="x", bufs=4) as xpool,
            tc.tile_pool(name="o", bufs=3) as opool,
            tc.tile_pool(name="ps", bufs=8, space="PSUM") as pspool,
        ):
            # Whole W.T resident in SBUF, split into KO chunks so early
            # matmuls only wait on the K-slices they read.
            w_tiles = []
            for ks in range(KO):
                wt = wpool.tile([P, D_OUT], MM_DT, tag=f"w{ks}")
                nc.sync.dma_start(wt[:], wT[ks * P:(ks + 1) * P, :])
                w_tiles.append(wt)

            for m in range(M_TILES):
                xt = xpool.tile([P, KO, P], MM_DT)
                nc.sync.dma_start(xt[:], xT3[:, :, m * P:(m + 1) * P])
                ot = opool.tile([P, D_OUT], mybir.dt.float32)
                for nb in range(NB):
                    ps = pspool.tile([P, 512], mybir.dt.float32)
                    for ks in range(KO):
                        nc.tensor.matmul(
                            ps[:],
                            lhsT=xt[:, ks, :],
                            rhs=w_tiles[ks][:, nb * 512:(nb + 1) * 512],
                            start=(ks == 0),
                            stop=(ks == KO - 1),
                        )
                    nc.any.tensor_copy(out=ot[:, nb * 512:(nb + 1) * 512], in_=ps[:])
                nc.sync.dma_start(y[m * P:(m + 1) * P, :], ot[:])

    nc.compile()
    return nc


def _get_nc(C: int):
    if C not in _nc_cache:
        _nc_cache[C] = _build_nc(C)
    return _nc_cache[C]


def _route(input_tokens, expert_assignments):
    """Host-side dispatch: group tokens by expert, pad to capacity."""
    a = np.asarray(expert_assignments)
    x = np.ascontiguousarray(np.asarray(input_tokens, dtype=np.float32))
    T = x.shape[0]
    order = np.argsort(a, kind="stable")
    counts = np.bincount(a.astype(np.int64), minlength=NUM_EXPERTS)
    starts = np.zeros(NUM_EXPERTS + 1, dtype=np.int64)
    np.cumsum(counts, out=starts[1:])
    C = max(P, int(-(-counts.max() // P)) * P)
    xs = x[order]  # [T, D_IN] sorted by expert
    xsT = np.ascontiguousarray(xs.T)  # [D_IN, T]
    return order, counts, starts, C, xsT


def kernel(input_tokens, weight, expert_assignments):
    order, counts, starts, C, xsT = _route(input_tokens, expert_assignments)
    w = np.asarray(weight, dtype=np.float32)
    T = xsT.shape[1]

    nc = _get_nc(C)
    in_maps = []
    for e in range(NUM_EXPERTS):
        s, cnt = int(starts[e]), int(counts[e])
        xTe = np.zeros((D_IN, C), dtype=np.float32)
        xTe[:, :cnt] = xsT[:, s:s + cnt]
        wTe = np.ascontiguousarray(w[e].T)  # [in, out]
        in_maps.append({"xT": xTe, "wT": wTe})

    res = run_bass_kernel_spmd(nc, in_maps, list(range(NUM_EXPERTS)))

    out = np.empty((T, D_OUT), dtype=np.float32)
    for e in range(NUM_EXPERTS):
        s, cnt = int(starts[e]), int(counts[e])
        out[order[s:s + cnt]] = res.results[e]["y"][:cnt]
    return out
